# revision 9
# baseline (speedup 1.0000x reference)
"""LSTMCell (B=65536, H=512) Bass/Tile kernel for 8 trn2 NeuronCores.

Data-parallel over batch: each core processes 8192 rows.
Per 128-row tile (all matmul inputs bf16, accum fp32):
  z = x + stm                      (GpSimd, f32 in -> bf16 out)
  zT chunks via PE transpose       (TensorE, bf16 -> PSUM)
  zT PSUM -> SBUF                  (ACT copy)
  gates in two PSUM halves A=(f,i) B=(o,g), 1024-wide matmuls,
  accumulated over 4 k-chunks      (TensorE)
  preA/preB = gates + bias         (DVE, drains PSUM half while PE
                                    fills the other half)
  sigmoid/tanh                     (ACT, bf16 out)
  prod = si*tg (DVE 2x bf16); c = sf + prod (DVE, f32 out)
  tc = tanh(c) (ACT); h = tc*so    (GpSimd, f32 out)
"""

import os
import sys

if "/opt/trn_rl_repo" not in sys.path:
    sys.path.insert(0, "/opt/trn_rl_repo")

import numpy as np

import concourse.bacc as bacc
import concourse.mybir as mybir
import concourse.tile as tile

N_CORES = 8
B, H = 65536, 512
B_CORE = B // N_CORES  # 8192
F32 = mybir.dt.float32
BF16 = mybir.dt.bfloat16
AF = mybir.ActivationFunctionType

NEFF_DUMP = "/tmp/lstm_kernel.neff"

# wt layout [128, 8192] bf16: block k (2048 cols) = [Wf_k | Wi_k | Wo_k | Wg_k],
# each 512 wide: wt[p, k*2048 + slot*512 + j] = W_slot[j, k*128 + p].
# bias layout [128, 2048] f32 = [bf | bi | bo | bg] replicated over partitions.

MM_W = 512  # matmul moving width (1 PSUM bank per instr; ISA max)


def build_module(b_core=B_CORE, n_cores=N_CORES):
    nc = bacc.Bacc(
        "TRN2",
        target_bir_lowering=False,
        debug=False,
        num_devices=n_cores,
    )
    x = nc.dram_tensor("x", [b_core, H], F32, kind="ExternalInput").ap()
    s = nc.dram_tensor("s", [b_core, H], F32, kind="ExternalInput").ap()
    wt = nc.dram_tensor("wt", [128, 8192], BF16, kind="ExternalInput").ap()
    bias = nc.dram_tensor("bias", [128, 2048], F32, kind="ExternalInput").ap()
    ident = nc.dram_tensor("ident", [128, 128], BF16, kind="ExternalInput").ap()
    out = nc.dram_tensor("out", [2, b_core, H], F32, kind="ExternalOutput").ap()

    n_tiles = b_core // 128

    with tile.TileContext(nc) as tc:
        with (
            tc.tile_pool(name="const", bufs=1) as cpool,
            tc.tile_pool(name="work", bufs=3) as pool,
            tc.tile_pool(name="pzt", bufs=2, space="PSUM") as pzt,
            tc.tile_pool(name="pga", bufs=1, space="PSUM") as pga,
            tc.tile_pool(name="pgb", bufs=1, space="PSUM") as pgb,
        ):
            id_sb = cpool.tile([128, 128], BF16)
            nc.sync.dma_start(out=id_sb[:], in_=ident[:])
            bias_sb = cpool.tile([128, 2048], F32)
            nc.sync.dma_start(out=bias_sb[:], in_=bias[:])
            wt_sb = cpool.tile([128, 8192], BF16)
            nc.sync.dma_start(out=wt_sb[:], in_=wt[:])

            # warm up the PE clock (p-state ramp) while weights stream in
            with tc.tile_pool(name="warm", bufs=1, space="PSUM") as wpool:
                scratch = wpool.tile([128, 128], BF16)
                for _ in range(48):
                    nc.tensor.transpose(scratch[:], id_sb[:], id_sb[:])

            # Software-pipelined 3 deep.  Iteration i runs:
            #   front(i):  loads, z, transposes, zt copy        (tile i)
            #   mid(i-1):  gate matmuls, bias, sigmoid/tanh,
            #              prod, c                               (tile i-1)
            #   tail(i-2): tanh(c), h                            (tile i-2)
            #   stores:    c lags 2 iters, h lags 3 iters
            # Ordered so no engine queues an op whose input isn't (nearly)
            # ready: ACT does tanh(c) of i-2 before the sigmoids of i-1,
            # DVE finishes with h of i-2 (input computed early this iter).
            stage_f = {}  # t -> zt_sb
            stage_m = {}  # t -> (sfi, so, c)
            stage_t = {}  # t -> (c, h)
            for i in range(n_tiles + 3):
                # ---- front(i) ----
                if i < n_tiles:
                    rows = slice(i * 128, (i + 1) * 128)
                    x_t = pool.tile([128, H], F32, tag="x")
                    nc.sync.dma_start(out=x_t[:], in_=x[rows, :])
                    s_t = pool.tile([128, H], F32, tag="s")
                    nc.sync.dma_start(out=s_t[:], in_=s[rows, :])
                    z_t = pool.tile([128, H], BF16, tag="z")
                    nc.gpsimd.tensor_add(z_t[:], x_t[:], s_t[:])

                    zt_ps = pzt.tile([128, H], BF16, tag="ztp")
                    for k in range(4):
                        nc.tensor.transpose(
                            zt_ps[:, k * 128 : (k + 1) * 128],
                            z_t[:, k * 128 : (k + 1) * 128],
                            id_sb[:],
                        )
                    zt_sb = pool.tile([128, H], BF16, tag="zt")
                    nc.scalar.copy(zt_sb[:], zt_ps[:])
                    stage_f[i] = zt_sb

                # ---- tail(i-2): ACT part first (input c is old) ----
                t2 = i - 2
                if t2 in stage_m:
                    sfi2, so2, c2 = stage_m.pop(t2)
                    tc_t = pool.tile([128, H], BF16, tag="tc")
                    nc.scalar.activation(tc_t[:], c2[:], AF.Tanh)
                    h_t = pool.tile([128, H], F32, tag="h")
                    nc.gpsimd.tensor_mul(h_t[:], tc_t[:], so2[:])
                    stage_t[t2] = (c2, h_t)

                # ---- mid(i-1) ----
                t1 = i - 1
                if 0 <= t1 < n_tiles:
                    zt = stage_f.pop(t1)
                    sfi = so = tg = None
                    for half, gpool in ((0, pga), (1, pgb)):
                        gp = gpool.tile([128, 1024], F32, tag=f"g{half}")
                        for k in range(4):
                            lhs = zt[:, k * 128 : (k + 1) * 128]
                            base = k * 2048 + half * 1024
                            for m0 in range(0, 1024, MM_W):
                                nc.tensor.matmul(
                                    gp[:, m0 : m0 + MM_W],
                                    lhs,
                                    wt_sb[:, base + m0 : base + m0 + MM_W],
                                    start=(k == 0),
                                    stop=(k == 3),
                                )
                        pre = pool.tile([128, 1024], F32, tag=f"pre{half}")
                        nc.vector.tensor_add(
                            pre[:], gp[:], bias_sb[:, half * 1024 : (half + 1) * 1024]
                        )
                        if half == 0:
                            sfi = pool.tile([128, 1024], BF16, tag="sfi")
                            nc.scalar.activation(sfi[:], pre[:], AF.Sigmoid)
                        else:
                            so = pool.tile([128, 512], BF16, tag="so")
                            nc.scalar.activation(so[:], pre[:, 0:512], AF.Sigmoid)
                            tg = pool.tile([128, 512], BF16, tag="tg")
                            nc.scalar.activation(tg[:], pre[:, 512:1024], AF.Tanh)

                    prod = pool.tile([128, 512], BF16, tag="prod")
                    nc.gpsimd.tensor_mul(prod[:], sfi[:, 512:1024], tg[:])
                    c_t = pool.tile([128, H], F32, tag="c")
                    nc.vector.tensor_add(c_t[:], sfi[:, 0:512], prod[:])
                    stage_m[t1] = (sfi, so, c_t)

                # ---- stores: c lags 2, h lags 3 ----
                ts_c = i - 2
                if ts_c in stage_t:
                    cc, _ = stage_t[ts_c]
                    prow = slice(ts_c * 128, (ts_c + 1) * 128)
                    nc.sync.dma_start(out=out[0, prow, :], in_=cc[:])
                ts_h = i - 3
                if ts_h in stage_t:
                    _, hh = stage_t.pop(ts_h)
                    prow = slice(ts_h * 128, (ts_h + 1) * 128)
                    nc.sync.dma_start(out=out[1, prow, :], in_=hh[:])

    nc.compile()
    return nc


def pack_inputs(inputs, short_term_memory, Wf, bf, Wi, bi, Wg, bg, Wo, bo):
    import ml_dtypes

    x = np.ascontiguousarray(np.asarray(inputs, np.float32))
    s = np.ascontiguousarray(np.asarray(short_term_memory, np.float32))
    Ws = [Wf, Wi, Wo, Wg]
    bs = [bf, bi, bo, bg]
    wt = np.empty((128, 8192), ml_dtypes.bfloat16)
    for slot, W in enumerate(Ws):
        Wt = np.asarray(W, np.float32).T  # [h, j] = W[j, h]
        # wt[p, k*2048 + slot*512 + j] = W[j, k*128+p]
        blk = Wt.reshape(4, 128, 512).astype(ml_dtypes.bfloat16)  # [k, p, j]
        for k in range(4):
            wt[:, k * 2048 + slot * 512 : k * 2048 + (slot + 1) * 512] = blk[k]
    bias = np.empty((128, 2048), np.float32)
    for slot, b in enumerate(bs):
        bias[:, slot * 512 : (slot + 1) * 512] = np.asarray(b, np.float32)[None, :]
    ident = np.eye(128, dtype=ml_dtypes.bfloat16)
    return {"x": x, "s": s, "wt": wt, "bias": bias, "ident": ident}


class Runner:
    """Compiles the module once and keeps a reusable jitted executor."""

    def __init__(self, nc=None, n_cores=N_CORES):
        import jax
        from concourse import bass2jax as b2j

        self.jax = jax
        self.n_cores = n_cores
        self.nc = nc or build_module(n_cores=n_cores)
        b2j.install_neuronx_cc_hook()

        # dump the final (renamed) NEFF so neuron-profile can pair it with NTFFs
        if not getattr(b2j, "_neff_dump_patched", False):
            orig = b2j.rename_neff_tensors_and_patch_header

            def _patched(neff_path, mapping):
                data = orig(neff_path, mapping)
                with open(NEFF_DUMP, "wb") as f:
                    f.write(data)
                return data

            b2j.rename_neff_tensors_and_patch_header = _patched
            b2j._neff_dump_patched = True

        from jax.experimental.shard_map import shard_map
        from jax.sharding import Mesh, NamedSharding, PartitionSpec

        part_name = (
            self.nc.partition_id_tensor.name if self.nc.partition_id_tensor else None
        )
        in_names, out_names, out_avals = [], [], []
        self.out_shapes = {}
        for alloc in self.nc.m.functions[0].allocations:
            if not isinstance(alloc, mybir.MemoryLocationSet):
                continue
            name = alloc.memorylocations[0].name
            if alloc.kind == "ExternalInput":
                if name != part_name:
                    in_names.append(name)
            elif alloc.kind == "ExternalOutput":
                out_names.append(name)
                shape = tuple(alloc.tensor_shape)
                dt = mybir.dt.np(alloc.dtype)
                out_avals.append(jax.core.ShapedArray(shape, dt))
                self.out_shapes[name] = (shape, dt)
        self.in_names, self.out_names = in_names, out_names
        nc_ref = self.nc

        bind_names = list(in_names) + list(out_names)
        if part_name is not None:
            bind_names.append(part_name)

        def _body(*args):
            operands = list(args)
            if part_name is not None:
                operands.append(b2j.partition_id_tensor())
            outs = b2j._bass_exec_p.bind(
                *operands,
                out_avals=tuple(out_avals),
                in_names=tuple(bind_names),
                out_names=tuple(out_names),
                lowering_input_output_aliases=(),
                sim_require_finite=False,
                sim_require_nnan=False,
                nc=nc_ref,
            )
            return tuple(outs)

        devices = jax.devices()[: self.n_cores]
        mesh = Mesh(np.asarray(devices), ("core",))
        spec = PartitionSpec("core")
        n_args = len(in_names) + len(out_names)
        self.sharding = NamedSharding(mesh, spec)
        self.fn = jax.jit(
            shard_map(
                _body,
                mesh=mesh,
                in_specs=(spec,) * n_args,
                out_specs=(spec,) * len(out_names),
                check_rep=False,
            ),
            keep_unused=True,
        )
        self._dev_args = None

    def stage(self, packed):
        """Transfer inputs (sharded/replicated as needed) to devices once."""
        jax = self.jax
        nc_n = self.n_cores
        args = []
        for name in self.in_names:
            a = packed[name]
            if name in ("x", "s"):
                glob = a  # already [B, H]; shard axis 0 into 8
            else:
                glob = np.concatenate([a] * nc_n, axis=0)  # replicate
            args.append(glob)
        for name in self.out_names:
            shape, dt = self.out_shapes[name]
            args.append(np.zeros((shape[0] * nc_n,) + shape[1:], dt))
        self._dev_args = [jax.device_put(a, self.sharding) for a in args]

    def execute(self):
        outs = self.fn(*self._dev_args)
        self.jax.block_until_ready(outs)
        return outs

    def run(self, packed):
        self.stage(packed)
        outs = self.execute()
        res = {}
        for name, arr in zip(self.out_names, outs):
            a = np.asarray(arr)  # [n_cores*d0, ...]
            shape, _ = self.out_shapes[name]
            res[name] = a.reshape((self.n_cores, shape[0]) + tuple(shape[1:]))
        return res


_RUNNER = None


def _get_runner():
    global _RUNNER
    if _RUNNER is None:
        _RUNNER = Runner()
    return _RUNNER


def kernel(**inputs):
    r = _get_runner()
    packed = pack_inputs(**inputs)
    res = r.run(packed)
    per_core = res["out"]  # [8, 2, 8192, 512]
    return np.ascontiguousarray(
        per_core.transpose(1, 0, 2, 3).reshape(2, B, H)
    )


if __name__ == "__main__":
    nc = build_module()
    print("module built + compiled OK")


# revision 11
# speedup vs baseline: 1.1558x; 1.1558x over previous
"""LSTMCell (B=65536, H=512) Bass/Tile kernel for 8 trn2 NeuronCores.

Data-parallel over batch: each core processes 8192 rows.
Per 128-row tile (all matmul inputs bf16, accum fp32):
  z = x + stm                      (GpSimd, f32 in -> bf16 out)
  zT chunks via PE transpose       (TensorE, bf16 -> PSUM)
  zT PSUM -> SBUF                  (ACT copy)
  gates in two PSUM halves A=(f,i) B=(o,g), 1024-wide matmuls,
  accumulated over 4 k-chunks      (TensorE)
  preA/preB = gates + bias         (DVE, drains PSUM half while PE
                                    fills the other half)
  sigmoid/tanh                     (ACT, bf16 out)
  prod = si*tg (DVE 2x bf16); c = sf + prod (DVE, f32 out)
  tc = tanh(c) (ACT); h = tc*so    (GpSimd, f32 out)
"""

import os
import sys

if "/opt/trn_rl_repo" not in sys.path:
    sys.path.insert(0, "/opt/trn_rl_repo")

import numpy as np

import concourse.bacc as bacc
import concourse.mybir as mybir
import concourse.tile as tile

N_CORES = 8
B, H = 65536, 512
B_CORE = B // N_CORES  # 8192
F32 = mybir.dt.float32
BF16 = mybir.dt.bfloat16
AF = mybir.ActivationFunctionType

NEFF_DUMP = "/tmp/lstm_kernel.neff"

# wt layout [128, 8192] bf16: block k (2048 cols) = [Wf_k | Wi_k | Wo_k | Wg_k],
# each 512 wide: wt[p, k*2048 + slot*512 + j] = W_slot[j, k*128 + p].
# bias layout [128, 2048] f32 = [bf | bi | bo | bg] replicated over partitions.

MM_W = 512  # matmul moving width (1 PSUM bank per instr; ISA max)


def build_module(b_core=B_CORE, n_cores=N_CORES):
    nc = bacc.Bacc(
        "TRN2",
        target_bir_lowering=False,
        debug=False,
        num_devices=n_cores,
    )
    x = nc.dram_tensor("x", [b_core, H], F32, kind="ExternalInput").ap()
    s = nc.dram_tensor("s", [b_core, H], F32, kind="ExternalInput").ap()
    wt = nc.dram_tensor("wt", [128, 8192], BF16, kind="ExternalInput").ap()
    bias = nc.dram_tensor("bias", [128, 2048], F32, kind="ExternalInput").ap()
    ident = nc.dram_tensor("ident", [128, 128], BF16, kind="ExternalInput").ap()
    out = nc.dram_tensor("out", [2, b_core, H], F32, kind="ExternalOutput").ap()

    n_tiles = b_core // 128

    with tile.TileContext(nc) as tc:
        with (
            tc.tile_pool(name="const", bufs=1) as cpool,
            tc.tile_pool(name="work", bufs=3) as pool,
            tc.tile_pool(name="pzt", bufs=1, space="PSUM") as pzt,
            tc.tile_pool(name="pga", bufs=2, space="PSUM") as pga,
            tc.tile_pool(name="pgb", bufs=1, space="PSUM") as pgb,
        ):
            id_sb = cpool.tile([128, 128], BF16)
            nc.sync.dma_start(out=id_sb[:], in_=ident[:])
            bias_sb = cpool.tile([128, 2048], F32)
            nc.sync.dma_start(out=bias_sb[:], in_=bias[:])
            wt_sb = cpool.tile([128, 8192], BF16)
            nc.sync.dma_start(out=wt_sb[:], in_=wt[:])

            # warm up the PE clock (p-state ramp) while weights stream in
            with tc.tile_pool(name="warm", bufs=1, space="PSUM") as wpool:
                scratch = wpool.tile([128, 128], BF16)
                for _ in range(48):
                    nc.tensor.transpose(scratch[:], id_sb[:], id_sb[:])

            # Software-pipelined 3 deep.  Iteration i runs:
            #   front(i):  loads, z, transposes, zt copy        (tile i)
            #   mid(i-1):  gate matmuls, bias, sigmoid/tanh,
            #              prod, c                               (tile i-1)
            #   tail(i-2): tanh(c), h                            (tile i-2)
            #   stores:    c lags 2 iters, h lags 3 iters
            # Ordered so no engine queues an op whose input isn't (nearly)
            # ready: ACT does tanh(c) of i-2 before the sigmoids of i-1,
            # DVE finishes with h of i-2 (input computed early this iter).
            stage_f = {}  # t -> zt_sb
            stage_m = {}  # t -> (sfi, so, c)
            stage_t = {}  # t -> (c, h)
            for i in range(n_tiles + 3):
                # ---- front(i) ----
                if i < n_tiles:
                    rows = slice(i * 128, (i + 1) * 128)
                    x_t = pool.tile([128, H], F32, tag="x")
                    nc.sync.dma_start(out=x_t[:], in_=x[rows, :])
                    s_t = pool.tile([128, H], F32, tag="s")
                    nc.sync.dma_start(out=s_t[:], in_=s[rows, :])
                    z_t = pool.tile([128, H], BF16, tag="z")
                    nc.gpsimd.tensor_add(z_t[:], x_t[:], s_t[:])

                    zt_ps = pzt.tile([128, H], BF16, tag="ztp")
                    for k in range(4):
                        nc.tensor.transpose(
                            zt_ps[:, k * 128 : (k + 1) * 128],
                            z_t[:, k * 128 : (k + 1) * 128],
                            id_sb[:],
                        )
                    zt_sb = pool.tile([128, H], BF16, tag="zt")
                    nc.scalar.copy(zt_sb[:], zt_ps[:])
                    stage_f[i] = zt_sb

                # ---- tail(i-2): ACT part first (input c is old) ----
                t2 = i - 2
                if t2 in stage_m:
                    sfi2, so2, c2 = stage_m.pop(t2)
                    tc_t = pool.tile([128, H], BF16, tag="tc")
                    nc.scalar.activation(tc_t[:], c2[:], AF.Tanh)
                    h_t = pool.tile([128, H], F32, tag="h")
                    nc.gpsimd.tensor_mul(h_t[:], tc_t[:], so2[:])
                    stage_t[t2] = (c2, h_t)

                # ---- mid(i-1) ----
                t1 = i - 1
                if 0 <= t1 < n_tiles:
                    zt = stage_f.pop(t1)
                    sfi = so = tg = None
                    for half, gpool in ((0, pga), (1, pgb)):
                        gp = gpool.tile([128, 1024], F32, tag=f"g{half}")
                        for k in range(4):
                            lhs = zt[:, k * 128 : (k + 1) * 128]
                            base = k * 2048 + half * 1024
                            for m0 in range(0, 1024, MM_W):
                                nc.tensor.matmul(
                                    gp[:, m0 : m0 + MM_W],
                                    lhs,
                                    wt_sb[:, base + m0 : base + m0 + MM_W],
                                    start=(k == 0),
                                    stop=(k == 3),
                                )
                        pre = pool.tile([128, 1024], F32, tag=f"pre{half}")
                        nc.vector.tensor_add(
                            pre[:], gp[:], bias_sb[:, half * 1024 : (half + 1) * 1024]
                        )
                        if half == 0:
                            sfi = pool.tile([128, 1024], BF16, tag="sfi")
                            nc.scalar.activation(sfi[:], pre[:], AF.Sigmoid)
                        else:
                            so = pool.tile([128, 512], BF16, tag="so")
                            nc.scalar.activation(so[:], pre[:, 0:512], AF.Sigmoid)
                            tg = pool.tile([128, 512], BF16, tag="tg")
                            nc.scalar.activation(tg[:], pre[:, 512:1024], AF.Tanh)

                    prod = pool.tile([128, 512], BF16, tag="prod")
                    nc.vector.tensor_mul(prod[:], sfi[:, 512:1024], tg[:])
                    c_t = pool.tile([128, H], F32, tag="c")
                    nc.vector.tensor_add(c_t[:], sfi[:, 0:512], prod[:])
                    stage_m[t1] = (sfi, so, c_t)

                # ---- stores: c lags 2, h lags 3 ----
                ts_c = i - 2
                if ts_c in stage_t:
                    cc, _ = stage_t[ts_c]
                    prow = slice(ts_c * 128, (ts_c + 1) * 128)
                    nc.sync.dma_start(out=out[0, prow, :], in_=cc[:])
                ts_h = i - 3
                if ts_h in stage_t:
                    _, hh = stage_t.pop(ts_h)
                    prow = slice(ts_h * 128, (ts_h + 1) * 128)
                    nc.sync.dma_start(out=out[1, prow, :], in_=hh[:])

    nc.compile()
    return nc


def pack_inputs(inputs, short_term_memory, Wf, bf, Wi, bi, Wg, bg, Wo, bo):
    import ml_dtypes

    x = np.ascontiguousarray(np.asarray(inputs, np.float32))
    s = np.ascontiguousarray(np.asarray(short_term_memory, np.float32))
    Ws = [Wf, Wi, Wo, Wg]
    bs = [bf, bi, bo, bg]
    wt = np.empty((128, 8192), ml_dtypes.bfloat16)
    for slot, W in enumerate(Ws):
        Wt = np.asarray(W, np.float32).T  # [h, j] = W[j, h]
        # wt[p, k*2048 + slot*512 + j] = W[j, k*128+p]
        blk = Wt.reshape(4, 128, 512).astype(ml_dtypes.bfloat16)  # [k, p, j]
        for k in range(4):
            wt[:, k * 2048 + slot * 512 : k * 2048 + (slot + 1) * 512] = blk[k]
    bias = np.empty((128, 2048), np.float32)
    for slot, b in enumerate(bs):
        bias[:, slot * 512 : (slot + 1) * 512] = np.asarray(b, np.float32)[None, :]
    ident = np.eye(128, dtype=ml_dtypes.bfloat16)
    return {"x": x, "s": s, "wt": wt, "bias": bias, "ident": ident}


class Runner:
    """Compiles the module once and keeps a reusable jitted executor."""

    def __init__(self, nc=None, n_cores=N_CORES):
        import jax
        from concourse import bass2jax as b2j

        self.jax = jax
        self.n_cores = n_cores
        self.nc = nc or build_module(n_cores=n_cores)
        b2j.install_neuronx_cc_hook()

        # dump the final (renamed) NEFF so neuron-profile can pair it with NTFFs
        if not getattr(b2j, "_neff_dump_patched", False):
            orig = b2j.rename_neff_tensors_and_patch_header

            def _patched(neff_path, mapping):
                data = orig(neff_path, mapping)
                with open(NEFF_DUMP, "wb") as f:
                    f.write(data)
                return data

            b2j.rename_neff_tensors_and_patch_header = _patched
            b2j._neff_dump_patched = True

        from jax.experimental.shard_map import shard_map
        from jax.sharding import Mesh, NamedSharding, PartitionSpec

        part_name = (
            self.nc.partition_id_tensor.name if self.nc.partition_id_tensor else None
        )
        in_names, out_names, out_avals = [], [], []
        self.out_shapes = {}
        for alloc in self.nc.m.functions[0].allocations:
            if not isinstance(alloc, mybir.MemoryLocationSet):
                continue
            name = alloc.memorylocations[0].name
            if alloc.kind == "ExternalInput":
                if name != part_name:
                    in_names.append(name)
            elif alloc.kind == "ExternalOutput":
                out_names.append(name)
                shape = tuple(alloc.tensor_shape)
                dt = mybir.dt.np(alloc.dtype)
                out_avals.append(jax.core.ShapedArray(shape, dt))
                self.out_shapes[name] = (shape, dt)
        self.in_names, self.out_names = in_names, out_names
        nc_ref = self.nc

        bind_names = list(in_names) + list(out_names)
        if part_name is not None:
            bind_names.append(part_name)

        def _body(*args):
            operands = list(args)
            if part_name is not None:
                operands.append(b2j.partition_id_tensor())
            outs = b2j._bass_exec_p.bind(
                *operands,
                out_avals=tuple(out_avals),
                in_names=tuple(bind_names),
                out_names=tuple(out_names),
                lowering_input_output_aliases=(),
                sim_require_finite=False,
                sim_require_nnan=False,
                nc=nc_ref,
            )
            return tuple(outs)

        devices = jax.devices()[: self.n_cores]
        mesh = Mesh(np.asarray(devices), ("core",))
        spec = PartitionSpec("core")
        n_args = len(in_names) + len(out_names)
        self.sharding = NamedSharding(mesh, spec)
        self.fn = jax.jit(
            shard_map(
                _body,
                mesh=mesh,
                in_specs=(spec,) * n_args,
                out_specs=(spec,) * len(out_names),
                check_rep=False,
            ),
            keep_unused=True,
        )
        self._dev_args = None

    def stage(self, packed):
        """Transfer inputs (sharded/replicated as needed) to devices once."""
        jax = self.jax
        nc_n = self.n_cores
        args = []
        for name in self.in_names:
            a = packed[name]
            if name in ("x", "s"):
                glob = a  # already [B, H]; shard axis 0 into 8
            else:
                glob = np.concatenate([a] * nc_n, axis=0)  # replicate
            args.append(glob)
        for name in self.out_names:
            shape, dt = self.out_shapes[name]
            args.append(np.zeros((shape[0] * nc_n,) + shape[1:], dt))
        self._dev_args = [jax.device_put(a, self.sharding) for a in args]

    def execute(self):
        outs = self.fn(*self._dev_args)
        self.jax.block_until_ready(outs)
        return outs

    def run(self, packed):
        self.stage(packed)
        outs = self.execute()
        res = {}
        for name, arr in zip(self.out_names, outs):
            a = np.asarray(arr)  # [n_cores*d0, ...]
            shape, _ = self.out_shapes[name]
            res[name] = a.reshape((self.n_cores, shape[0]) + tuple(shape[1:]))
        return res


_RUNNER = None


def _get_runner():
    global _RUNNER
    if _RUNNER is None:
        _RUNNER = Runner()
    return _RUNNER


def kernel(**inputs):
    r = _get_runner()
    packed = pack_inputs(**inputs)
    res = r.run(packed)
    per_core = res["out"]  # [8, 2, 8192, 512]
    return np.ascontiguousarray(
        per_core.transpose(1, 0, 2, 3).reshape(2, B, H)
    )


if __name__ == "__main__":
    nc = build_module()
    print("module built + compiled OK")
